# revision 25
# baseline (speedup 1.0000x reference)
"""Bass/Tile kernel for nn_EnergyDipolesMACE on 8 TRN2 NeuronCores (v2).

Host (index/layout prep): sort edges by destination window, shard destination
nodes across cores (1024 each, 8 windows of 128), pad window edge lists to 18
chunks of 128; pre-gather per-edge positions (src+dst) and source-species
one-hots; pack int16 gather indices for the one remaining device gather.

Device per core:
  setup: edge geometry (Y, radial basis) once; radial MLP for BOTH iterations
  in one stacked pass (128-wide PE); one-hot scatter blocks; per-edge h0 via
  species-one-hot matmuls; broadcast-Y materialized once (Pool) so per-iter
  products run in DVE bf16 2x mode.
  per iteration: px = s3 @ R3 per chunk; msg = (Ybc*hs)*px in bf16; scatter
  via one-hot matmuls into PSUM; window-batched node phase (padded-Wmix
  accumulate trick + [128,128] block transposes); readouts via
  scalar_tensor_tensor accumulation (no per-window transposes).
  Between iterations: AllGather of the scalar channel; its output buffer is
  the gather table for iter-1 source features (single SWDGE gather).
"""
import math, os
import numpy as np

import concourse.bacc as bacc
import concourse.bass as bass
import concourse.tile as tile
from concourse import mybir

try:
    import ml_dtypes
    BF16_NP = ml_dtypes.bfloat16
except Exception:  # pragma: no cover
    BF16_NP = np.float32

# allow 128B gather payloads (row stride stays 256B-aligned; probed on HW)
import textwrap as _tw, inspect as _ins
_gsrc = _tw.dedent(_ins.getsource(bass.BassGpSimd.dma_gather))
if "% 256 == 0" in _gsrc:
    _gsrc = _gsrc.replace("elem_size_bytes > 0 and elem_size_bytes % 256 == 0",
                          "elem_size_bytes > 0 and elem_size_bytes % 128 == 0")
    _gns = dict(bass.__dict__)
    exec(compile(_gsrc, "<patched_dma_gather>", "exec"), _gns)
    bass.BassGpSimd.dma_gather = _gns["dma_gather"]

f32 = mybir.dt.float32
f32r = mybir.dt.float32r
bf16 = mybir.dt.bfloat16
i16 = mybir.dt.int16
AF = mybir.ActivationFunctionType
ALU = mybir.AluOpType

N, E, C, Z, G, NB, NSH = 8192, 131072, 32, 10, 16, 8, 9
R_MAX, P_CUT, AVG_NEIGH = 5.0, 5, 16.0
LMAP = np.array([0, 1, 1, 1, 2, 2, 2, 2, 2])
NCORES = 8
NPC = N // NCORES
WIN = 128
WPC = NPC // WIN               # 8 windows/core
W_CAP = 18                     # chunks per window
CHUNK = 128
L_PAD = WPC * W_CAP * CHUNK    # 18432
NCHUNKS = L_PAD // CHUNK       # 144
SC = 384                       # MLP superchunk (3 chunks; 6 per window)
NSC = L_PAD // SC              # 48
NQ = 4                         # SWDGE queues
IDX_COLS = L_PAD // 16         # 1152
MC = NSH * C                   # 288
S3, S5, S15 = 3.0 ** 0.5, 5.0 ** 0.5, 15.0 ** 0.5
PREF = (2.0 / R_MAX) ** 0.5
PCF = float(P_CUT)
ENV_A = -(PCF + 1.0) * (PCF + 2.0) / 2.0
ENV_B = PCF * (PCF + 2.0)
ENV_C = -PCF * (PCF + 1.0) / 2.0
TWO_PI = 2 * math.pi


def host_prep(inputs):
    snd = np.asarray(inputs["edge_index"])[0].astype(np.int64)
    rcv = np.asarray(inputs["edge_index"])[1].astype(np.int64)
    batch = np.asarray(inputs["batch"]).astype(np.int64)
    positions = np.asarray(inputs["positions"], np.float32)
    node_attrs = np.asarray(inputs["node_attrs"], np.float32)
    charges = np.asarray(inputs["charges"], np.float32)

    order = np.argsort(rcv, kind="stable")
    snd_s, rcv_s = snd[order], rcv[order]
    win_id = rcv_s // WIN
    counts = np.bincount(win_id, minlength=N // WIN)
    assert counts.max() <= W_CAP * CHUNK, f"window overflow: {counts.max()}"

    iota = np.tile(np.arange(128, dtype=np.float32)[None, :], (128, 1))
    ident = np.eye(128, dtype=np.float32)
    nvec = np.tile((np.arange(1, NB + 1, dtype=np.float32) * math.pi / R_MAX)[None, :],
                   (128, 1))
    # stacked MLP weights: iteration 0 -> hidden cols/rows 0:64, iter 1 -> 64:128
    R0 = [np.asarray(inputs["R0"][i], np.float32) for i in range(2)]
    R1 = [np.asarray(inputs["R1"][i], np.float32) for i in range(2)]
    R2 = [np.asarray(inputs["R2"][i], np.float32) for i in range(2)]
    R0s2 = np.concatenate(R0, 1)                      # [8, 128]
    R1s2 = np.zeros((128, 128), np.float32)
    R1s2[0:64, 0:64] = R1[0]; R1s2[64:128, 64:128] = R1[1]
    R2s2 = np.zeros((128, 128), np.float32)
    R2s2[0:64, 0:64] = R2[0]; R2s2[64:128, 64:128] = R2[1]
    R3e2 = np.concatenate([np.asarray(inputs["R3"][i], np.float32)
                           .reshape(64, 3, C)[:, LMAP, :].reshape(64, MC)
                           for i in range(2)], 0)     # [128, 288]
    # Wmix padded groups, pre-scaled by 1/AVG_NEIGH (A = segsum/16 fold)
    # group g covers m in [4g, 4g+4); stationary [32, 128] with m-block at
    # cols 32*(m%4).. so 4 matmuls accumulate into one [128, 512] PSUM tile.
    WmixP = np.zeros((C, 2, NSH, 96), np.float32)
    WscP = np.zeros((C, NSH, 96), np.float32)
    for i in range(2):
        Wm = np.asarray(inputs["W_mix"][i], np.float32)[LMAP] / AVG_NEIGH  # [9,C,C]
        for m in range(NSH):
            WmixP[:, i, m, 32 * (m % 3):32 * (m % 3) + 32] = Wm[m]
    Ws1 = np.asarray(inputs["W_sc"][1], np.float32)[LMAP]
    for m in range(NSH):
        WscP[:, m, 32 * (m % 3):32 * (m % 3) + 32] = Ws1[m]
    Wemb = np.asarray(inputs["W_embed"], np.float32)          # [Z, C]
    AE = np.asarray(inputs["atomic_energies"], np.float32)
    Wsc00 = np.asarray(inputs["W_sc"][0], np.float32)[0]      # [C, C] (l=0)
    # per-own-node bundle: w123 both iters (192) | e0 (1) | sc0 = Wemb@Wsc00 (32)
    Wp = np.stack([np.asarray(inputs[f"Wp{j}"], np.float32) for j in (1, 2, 3)], 1)
    rhsN = np.concatenate(
        [Wp[0].transpose(1, 0, 2).reshape(Z, 96), Wp[1].transpose(1, 0, 2).reshape(Z, 96),
         AE[:, None], Wemb @ Wsc00], 1)                        # [10, 225]
    Wh = np.asarray(inputs["Wh"], np.float32)                  # [C, 16]
    wE2 = np.asarray(inputs["wE2"], np.float32)[:, None]       # [16, 1]
    wE1bc = np.tile(np.asarray(inputs["wE1"], np.float32)[None, :], (128, 1))
    wE2bc = np.tile(np.asarray(inputs["wE2"], np.float32)[None, :], (128, 1))
    wD1bc = np.tile(np.tile(np.asarray(inputs["wD1"], np.float32), 3)[None, :], (128, 1))
    wD2bc = np.tile(np.tile(np.asarray(inputs["wD2"], np.float32), 3)[None, :], (128, 1))

    WmixP = np.tile(WmixP, (3, 1, 1, 1))   # replicate at bases 0/32/64
    WscP = np.tile(WscP, (3, 1, 1))
    Wh3 = np.tile(Wh, (3, 1))
    shared = dict(iota=iota, ident=ident, nvec=nvec, R0s2=R0s2, R1s2=R1s2,
                  R2s2=R2s2, R3e2=R3e2, WmixP=WmixP, WscP=WscP, WembE=Wemb,
                  rhsN=rhsN, Wh=Wh3, wE2=wE2, wE1bc=wE1bc, wE2bc=wE2bc,
                  wD1bc=wD1bc, wD2bc=wD2bc)

    species = np.argmax(node_attrs, 1)  # one-hot input
    in_maps = []
    for k in range(NCORES):
        snd_pad = np.zeros(L_PAD, np.int64)
        rcv_pad = np.zeros(L_PAD, np.int64)
        rcv_loc = np.full(L_PAD, -1000.0, np.float32)
        for w in range(WPC):
            gw = k * WPC + w
            sel = win_id == gw
            cnt = int(counts[gw])
            base = w * W_CAP * CHUNK
            snd_pad[base:base + cnt] = snd_s[sel]
            rcv_pad[base:base + cnt] = rcv_s[sel]
            rcv_loc[base:base + cnt] = (rcv_s[sel] - gw * WIN).astype(np.float32)

        def wrap_idx(a):
            w16 = a.astype(np.int16).reshape(IDX_COLS, 16).T
            return np.tile(w16, (8, 1)).copy()

        own = slice(k * NPC, (k + 1) * NPC)
        m = dict(shared)
        m["gsnd"] = wrap_idx(snd_pad)
        m["rcvloc"] = np.ascontiguousarray(rcv_loc.reshape(NCHUNKS, CHUNK).T)
        # pre-gathered per-edge geometry inputs, chunk-major [128, 144, 3]
        ps = positions[snd_pad]                          # [L, 3]
        pr = positions[rcv_pad]
        m["posS"] = np.ascontiguousarray(
            ps.reshape(NCHUNKS, CHUNK, 3).transpose(1, 0, 2))
        m["posR"] = np.ascontiguousarray(
            pr.reshape(NCHUNKS, CHUNK, 3).transpose(1, 0, 2))
        # source species one-hot, transposed [10, L_PAD] (bf16)
        spc = np.zeros((Z, L_PAD), np.float32)
        spc[species[snd_pad], np.arange(L_PAD)] = 1.0
        m["spcT"] = spc.astype(BF16_NP)
        m["naTo"] = np.ascontiguousarray(node_attrs[own].T)   # [10, 1024]
        goh = np.zeros((NPC, G), np.float32)
        goh[np.arange(NPC), batch[own]] = 1.0
        m["goh"] = np.ascontiguousarray(goh.reshape(WPC, 128, G).transpose(1, 0, 2))
        m["qown"] = np.ascontiguousarray(charges[own].reshape(WPC, 128).T)
        m["posown"] = np.ascontiguousarray(
            positions[own].reshape(WPC, 128, 3).transpose(1, 0, 2))
        in_maps.append(m)
    return in_maps, {}


CONST_SPECS = dict(
    iota=([128, 128], f32), ident=([128, 128], f32), nvec=([128, NB], f32),
    R0s2=([NB, 128], f32), R1s2=([128, 128], f32), R2s2=([128, 128], f32),
    R3e2=([128, MC], f32), WmixP=([96, 2, NSH, 96], f32), WscP=([96, NSH, 96], f32),
    WembE=([Z, C], f32), rhsN=([Z, 225], f32), Wh=([96, 16], f32),
    wE2=([16, 1], f32), wE1bc=([128, C], f32), wE2bc=([128, 16], f32),
    wD1bc=([128, 96], f32),
    wD2bc=([128, 96], f32),
    gsnd=([128, IDX_COLS], i16), rcvloc=([128, NCHUNKS], f32),
    posS=([128, NCHUNKS, 3], f32), posR=([128, NCHUNKS, 3], f32),
    naTo=([Z, NPC], f32), goh=([128, WPC, G], f32), qown=([128, WPC], f32),
    posown=([128, WPC, 3], f32),
)
DRAM_ONLY_SPECS = dict(spcT=([Z, L_PAD], bf16))
INPUT_SPECS = {**CONST_SPECS, **DRAM_ONLY_SPECS}


def build_nc(num_devices=NCORES, sim_safe=False, use_f32r=True, phases=99, repeat=1):
    nc = bacc.Bacc("TRN2", target_bir_lowering=False, debug=False,
                   num_devices=num_devices, num_swdge_queues=NQ)
    inp = {name: nc.dram_tensor(name, shape, dt, kind="ExternalInput")
           for name, (shape, dt) in INPUT_SPECS.items()}
    y_out = nc.dram_tensor("y", [G, 4], f32, kind="ExternalOutput")
    oh_dram = nc.dram_tensor("ohd", [NCHUNKS, 128, 128], bf16, kind="Internal")
    agin = nc.dram_tensor("agin", [NPC, 128], bf16, kind="Internal")
    agout = nc.dram_tensor("agout", [N, 128], bf16, kind="Internal",
                           addr_space="Shared")

    def silu(out_ap, in_ap, pool, tag="siltmp"):
        if not sim_safe:
            nc.scalar.activation(out_ap, in_ap, AF.Silu)
        else:
            sg = pool.tile(list(out_ap.shape), f32, tag=tag)
            nc.scalar.activation(sg[:], in_ap, AF.Sigmoid)
            nc.vector.tensor_tensor(out_ap, in_ap, sg[:], ALU.mult)

    NQC = NCHUNKS // NQ
    GNI = 768
    GCH = GNI // 128
    GCALLS = L_PAD // NQ // GNI

    def gather(dst_tile, src_dram, idx_tile, ncol=C):
        for c in range(GCALLS):
            for q in range(NQ):
                b = q * NQC + c * GCH
                nc.gpsimd.dma_gather(
                    out_ap=dst_tile[:, b:b + GCH, :],
                    in_ap=src_dram.ap()[:, 0:ncol],
                    idxs_ap=idx_tile[:, b * 8:(b + GCH) * 8],
                    num_idxs=GNI, num_idxs_reg=GNI,
                    elem_size=ncol, elem_step=128, queue_num=q)

    with tile.TileContext(nc) as tc:
        with tc.tile_pool(name="const", bufs=1) as cst, \
             tc.tile_pool(name="big", bufs=1) as big, \
             tc.tile_pool(name="pms", bufs=2, space="PSUM") as pms:

            sb = {}
            for name, (shape, dt) in CONST_SPECS.items():
                t = cst.tile(shape, dt, tag=f"c_{name}")
                nc.sync.dma_start(out=t[:], in_=inp[name].ap())
                sb[name] = t
            # bf16/f32r conversions of weights
            identb = cst.tile([128, 128], bf16, tag="identb")
            nc.scalar.activation(identb[:], sb["ident"][:], AF.Copy)
            R0b = cst.tile([NB, 128], bf16, tag="R0b")
            nc.scalar.activation(R0b[:], sb["R0s2"][:], AF.Copy)
            R1r = cst.tile([128, 128], f32r, tag="R1r")
            nc.scalar.activation(R1r[:], sb["R1s2"][:], AF.Copy)
            R2r = cst.tile([128, 128], f32r, tag="R2r")
            nc.scalar.activation(R2r[:], sb["R2s2"][:], AF.Copy)
            R3eb = cst.tile([128, MC], bf16, tag="R3eb")
            nc.scalar.activation(R3eb[:], sb["R3e2"][:], AF.Copy)
            wE2b = cst.tile([16, 1], bf16, tag="wE2b")
            nc.scalar.activation(wE2b[:], sb["wE2"][:], AF.Copy)
            WscPb = cst.tile([96, NSH, 96], bf16, tag="WscPb")
            nc.scalar.activation(WscPb[:], sb["WscP"][:], AF.Copy)
            WembEb = cst.tile([Z, C], bf16, tag="WembEb")
            nc.scalar.activation(WembEb[:], sb["WembE"][:], AF.Copy)
            Ybc = big.tile([128, NCHUNKS, NSH, 8], bf16, tag="Ybc")
            s3b = big.tile([128, L_PAD], bf16, tag="s3b")
            hs_sb = big.tile([128, NCHUNKS, 64], bf16, tag="hs")
            A2 = big.tile([128, WPC, MC], f32, tag="A2")
            hN = big.tile([128, WPC, MC], f32, tag="hN")
            hT0 = big.tile([96, 3, 128], f32, tag="hT0")   # [(3w,32c), grp, 128n]
            hTf = big.tile([96, 3, WPC, 128], bf16, tag="hTf")  # h transposed grps
            w123 = big.tile([128, WPC, 2, 96], f32, tag="w123")
            sc0n = big.tile([128, WPC, C], f32, tag="sc0n")
            vals = big.tile([128, WPC, 8], f32, tag="vals")
            e2n = big.tile([128, WPC], f32, tag="e2n")

            for _rep in range(repeat):
              # ---------------- setup ----------------
              with tc.tile_pool(name="st", bufs=2) as st, \
                   tc.tile_pool(name="stA", bufs=1) as stA, \
                   tc.tile_pool(name="pst", bufs=2, space="PSUM") as pst:
                spc = stA.tile([Z, L_PAD], bf16, tag="spc")
                nc.sync.dma_start(out=spc[:], in_=inp["spcT"].ap())
                # own-node bundle: w123 | e0 | sc0
                for w in range(WPC):
                    pw = pst.tile([128, 225], f32, tag="pw")
                    nc.tensor.matmul(pw[:], sb["naTo"][:, w * 128:(w + 1) * 128],
                                     sb["rhsN"][:], start=True, stop=True)
                    nc.scalar.activation(
                        w123[:, w, :, :].rearrange("p a b -> p (a b)"),
                        pw[:, 0:192], AF.Copy)
                    nc.scalar.activation(vals[:, w, 0].unsqueeze(1),
                                         pw[:, 192:193], AF.Copy)
                    nc.scalar.activation(sc0n[:, w, :], pw[:, 193:225], AF.Copy)
                # per-edge h0 from species one-hots
                for g in range(NCHUNKS):
                    ph = pst.tile([128, C], f32, tag="ph")
                    nc.tensor.matmul(ph[:], spc[:, g * 128:(g + 1) * 128],
                                     WembEb[:], start=True, stop=True)
                    nc.scalar.activation(hs_sb[:, g, 0:C], ph[:], AF.Copy)
                # one-hot blocks -> DRAM
                for w in range(WPC):
                    ohb = st.tile([128, W_CAP, 128], bf16, tag="ohb")
                    for j in range(W_CAP):
                        nc.vector.tensor_scalar(
                            ohb[:, j, :], sb["iota"][:],
                            sb["rcvloc"][:, w * W_CAP + j].unsqueeze(1),
                            None, ALU.is_equal)
                    nc.sync.dma_start(
                        out=oh_dram.ap()[w * W_CAP:(w + 1) * W_CAP]
                            .rearrange("j p c -> p j c"),
                        in_=ohb[:])

              if phases >= 2:
               with tc.tile_pool(name="rbts", bufs=1) as rbp:
                rb8 = rbp.tile([NB, L_PAD], bf16, tag="rb8")
                with tc.tile_pool(name="geo", bufs=1) as gsc, \
                     tc.tile_pool(name="pgeo", bufs=2, space="PSUM") as pgs:
                    Ysb = gsc.tile([128, NCHUNKS, NSH], f32, tag="Y")
                    geo = gsc.tile([128, NCHUNKS, 14], f32, tag="geo")
                    vec, sq = geo[:, :, 0:3], geo[:, :, 3:6]
                    r2, r_, rinv = geo[:, :, 6], geo[:, :, 7], geo[:, :, 8]
                    u = geo[:, :, 9:12]
                    t0, t1 = geo[:, :, 12], geo[:, :, 13]
                    BC = [128, NCHUNKS, 3]
                    nc.vector.tensor_tensor(vec, sb["posR"][:], sb["posS"][:],
                                            ALU.subtract)
                    nc.vector.tensor_tensor(sq, vec, vec, ALU.mult)
                    nc.vector.tensor_reduce(r2.unsqueeze(2), sq,
                                            mybir.AxisListType.X, ALU.add)
                    nc.vector.tensor_scalar_add(r2.unsqueeze(2), r2.unsqueeze(2), 1e-12)
                    nc.scalar.activation(r_.unsqueeze(2), r2.unsqueeze(2), AF.Sqrt)
                    nc.vector.reciprocal(rinv.unsqueeze(2), r_.unsqueeze(2))
                    nc.vector.tensor_tensor(u, vec, rinv.unsqueeze(2).broadcast_to(BC),
                                            ALU.mult)
                    ux = u[:, :, 0].unsqueeze(2)
                    uy = u[:, :, 1].unsqueeze(2)
                    uz = u[:, :, 2].unsqueeze(2)
                    nc.vector.memset(Ysb[:, :, 0].unsqueeze(2), 1.0)
                    nc.scalar.activation(Ysb[:, :, 1:4], u, AF.Copy, scale=S3)
                    nc.vector.scalar_tensor_tensor(Ysb[:, :, 4].unsqueeze(2), ux, S15,
                                                   uy, ALU.mult, ALU.mult)
                    nc.vector.scalar_tensor_tensor(Ysb[:, :, 5].unsqueeze(2), uy, S15,
                                                   uz, ALU.mult, ALU.mult)
                    nc.vector.tensor_tensor(t0.unsqueeze(2), uz, uz, ALU.mult)
                    nc.scalar.activation(Ysb[:, :, 6].unsqueeze(2), t0.unsqueeze(2),
                                         AF.Copy, scale=3.0 * S5 / 2.0, bias=-S5 / 2.0)
                    nc.vector.scalar_tensor_tensor(Ysb[:, :, 7].unsqueeze(2), ux, S15,
                                                   uz, ALU.mult, ALU.mult)
                    nc.vector.tensor_tensor(t0.unsqueeze(2), ux, uy, ALU.add)
                    nc.vector.tensor_tensor(t1.unsqueeze(2), ux, uy, ALU.subtract)
                    nc.vector.scalar_tensor_tensor(Ysb[:, :, 8].unsqueeze(2),
                                                   t0.unsqueeze(2), S15 / 2.0,
                                                   t1.unsqueeze(2), ALU.mult, ALU.mult)
                    # materialize 8-wide c-repeated Y (bf16) on Pool; the
                    # product reads it via a stride-0 repeat AP at 2x mode
                    for q4 in range(4):
                        qs = q4 * (NCHUNKS // 4)
                        nc.gpsimd.tensor_copy(
                            Ybc[:, qs:qs + NCHUNKS // 4, :, :],
                            Ysb[:, qs:qs + NCHUNKS // 4, :].unsqueeze(3)
                                .broadcast_to([128, NCHUNKS // 4, NSH, 8]))
                    # radial basis
                    rbw = gsc.tile([128, NCHUNKS, NB], f32, tag="rbw")
                    BC8 = [128, NCHUNKS, NB]
                    nc.vector.tensor_tensor(rbw[:], r_.unsqueeze(2).broadcast_to(BC8),
                                            sb["nvec"].unsqueeze(1).broadcast_to(BC8),
                                            ALU.mult)
                    rmsk = gsc.tile([128, NCHUNKS, NB], f32, tag="rmsk")
                    rki = gsc.tile([128, NCHUNKS, NB], mybir.dt.int32, tag="rki")
                    nc.vector.tensor_scalar(rmsk[:], rbw[:], 1.0 / TWO_PI, None,
                                            ALU.mult)
                    nc.vector.tensor_copy(rki[:], rmsk[:])
                    nc.vector.tensor_copy(rmsk[:], rki[:])
                    nc.vector.scalar_tensor_tensor(rbw[:], rmsk[:], -TWO_PI, rbw[:],
                                                   ALU.mult, ALU.add)
                    nc.vector.tensor_scalar(rmsk[:], rbw[:], math.pi, None, ALU.is_gt)
                    nc.vector.scalar_tensor_tensor(rbw[:], rmsk[:], -TWO_PI, rbw[:],
                                                   ALU.mult, ALU.add)
                    nc.vector.tensor_scalar(rbw[:], rbw[:], math.pi, None, ALU.min)
                    nc.vector.tensor_scalar(rbw[:], rbw[:], -math.pi, None, ALU.max)
                    nc.scalar.activation(rbw[:], rbw[:], AF.Sin)
                    xx = t0.unsqueeze(2)
                    nc.vector.tensor_scalar(xx, r_.unsqueeze(2), 1.0 / R_MAX, None,
                                            ALU.mult)
                    x2 = t1.unsqueeze(2)
                    nc.vector.tensor_tensor(x2, xx, xx, ALU.mult)
                    x4 = geo[:, :, 3].unsqueeze(2)
                    nc.vector.tensor_tensor(x4, x2, x2, ALU.mult)
                    x5 = geo[:, :, 4].unsqueeze(2)
                    nc.vector.tensor_tensor(x5, x4, xx, ALU.mult)
                    q1 = geo[:, :, 5].unsqueeze(2)
                    nc.scalar.activation(q1, xx, AF.Copy, scale=ENV_C, bias=ENV_B)
                    q2 = t1.unsqueeze(2)
                    nc.vector.tensor_tensor(q2, q1, xx, ALU.mult)
                    nc.vector.tensor_scalar_add(q2, q2, ENV_A)
                    env = r2.unsqueeze(2)
                    nc.vector.tensor_tensor(env, x5, q2, ALU.mult)
                    nc.vector.tensor_scalar_add(env, env, 1.0)
                    mlt = geo[:, :, 3].unsqueeze(2)
                    nc.vector.tensor_scalar(mlt, xx, 1.0, None, ALU.is_lt)
                    nc.vector.tensor_tensor(env, env, mlt, ALU.mult)
                    wfac = geo[:, :, 4].unsqueeze(2)
                    nc.vector.scalar_tensor_tensor(wfac, rinv.unsqueeze(2), PREF, env,
                                                   ALU.mult, ALU.mult)
                    nc.vector.tensor_tensor(rbw[:], rbw[:],
                                            wfac.broadcast_to(BC8), ALU.mult)
                    for t4 in range(NCHUNKS // 4):
                        ptr = pgs.tile([NB, 512], f32, tag="ptr")
                        for j in range(4):
                            g = t4 * 4 + j
                            nc.tensor.transpose(ptr[:, j * 128:(j + 1) * 128],
                                                rbw[:, g, :], sb["ident"][:])
                        nc.scalar.activation(rb8[:, t4 * 512:(t4 + 1) * 512],
                                             ptr[:], AF.Copy)

                # stacked radial MLP for both iterations (once)
                with tc.tile_pool(name="mlp", bufs=2) as mp, \
                     tc.tile_pool(name="pmlp", bufs=2, space="PSUM") as pmlp:
                    for sci in range(NSC):
                        ee = sci * SC
                        p1 = pmlp.tile([128, SC], f32, tag="p1")
                        nc.tensor.matmul(p1[:], R0b[:], rb8[:, ee:ee + SC],
                                         start=True, stop=True)
                        s1 = mp.tile([128, SC], f32r, tag="s1")
                        silu(s1[:], p1[:], mp)
                        p2 = pmlp.tile([128, SC], f32, tag="p2")
                        nc.tensor.matmul(p2[:], R1r[:], s1[:], start=True, stop=True)
                        s2 = mp.tile([128, SC], f32r, tag="s2")
                        silu(s2[:], p2[:], mp)
                        p3 = pmlp.tile([128, SC], f32, tag="p3")
                        nc.tensor.matmul(p3[:], R2r[:], s2[:], start=True, stop=True)
                        silu(s3b[:, ee:ee + SC], p3[:], mp)

              # ---------------- iterations ----------------
              with tc.tile_pool(name="wk", bufs=3) as wk, \
                   tc.tile_pool(name="nd1", bufs=1) as nd1, \
                   tc.tile_pool(name="nd", bufs=2) as ndp, \
                   tc.tile_pool(name="px", bufs=4, space="PSUM") as pxp, \
                   tc.tile_pool(name="pa", bufs=2, space="PSUM") as pap:
                  niter = 2 if phases >= 4 else (1 if phases >= 3 else 0)
                  for it in range(niter):
                      # message + scatter, per window
                      for w in range(WPC):
                          ohw = wk.tile([128, W_CAP, 128], bf16, tag="ohw")
                          nc.sync.dma_start(
                              out=ohw[:],
                              in_=oh_dram.ap()[w * W_CAP:(w + 1) * W_CAP]
                                  .rearrange("j p c -> p j c"))
                          pA = pap.tile([128, MC], f32, tag="pA")
                          for s6 in range(6):
                              g3 = w * W_CAP + s6 * 3
                              yhg = wk.tile([128, 3, NSH, C], bf16, tag="yh")
                              pxg = wk.tile([128, 3, MC], bf16, tag="pxg")
                              for j in range(3):
                                  g = g3 + j
                                  yeng = (nc.gpsimd if g % 2 == 1 else nc.vector)
                                  yeng.tensor_tensor(
                                      yhg[:, j, :, :].rearrange(
                                          "p m (a b) -> p m a b", a=4),
                                      Ybc[:, g, :, :].unsqueeze(2)
                                          .broadcast_to([128, NSH, 4, 8]),
                                      hs_sb[:, g, 0:C].rearrange(
                                          "p (a b) -> p a b", a=4).unsqueeze(1)
                                          .broadcast_to([128, NSH, 4, 8]),
                                      ALU.mult)
                                  px = pxp.tile([128, MC], f32, tag="px")
                                  nc.tensor.matmul(
                                      px[:],
                                      s3b[it * 64:(it + 1) * 64,
                                          g * 128:(g + 1) * 128],
                                      R3eb[it * 64:(it + 1) * 64, :],
                                      start=True, stop=True)
                                  nc.scalar.activation(pxg[:, j, :], px[:],
                                                       AF.Copy)
                              msgg = wk.tile([128, 3, MC], bf16, tag="msgg")
                              eng = (nc.gpsimd if (s6 % 2 == 0 and
                                     os.environ.get("KPOOLMSG", "1")) else nc.vector)
                              eng.tensor_tensor(
                                  msgg[:],
                                  yhg[:].rearrange("p t m c -> p t (m c)"),
                                  pxg[:], ALU.mult)
                              for j in range(3):
                                  nc.tensor.matmul(pA[:], ohw[:, s6 * 3 + j, :],
                                                   msgg[:, j, :],
                                                   start=(s6 == 0 and j == 0),
                                                   stop=(s6 == 5 and j == 2))
                          nc.scalar.activation(A2[:, w, :], pA[:], AF.Copy)

                      KILVL = int(os.environ.get("KILVL", "9"))
                      # ---- node phase, batched over windows ----
                      # transpose A: 3 [128,96] blocks/window -> AT [96, grp, w, n]
                      if KILVL < 2:
                          continue
                      AT = nd1.tile([96, 3, WPC, 128], f32, tag="AT")
                      for w in range(WPC):
                          pmt = pms.tile([128, 512], f32, tag="pm")
                          ptA = pmt[0:96, 0:384]
                          for grp in range(3):
                              nc.tensor.transpose(
                                  ptA[:, grp * 128:(grp + 1) * 128],
                                  A2[:, w, grp * 96:(grp + 1) * 96],
                                  sb["ident"][:])
                          for grp in range(3):
                              nc.scalar.activation(
                                  AT[:, grp, w, :],
                                  ptA[:, grp * 128:(grp + 1) * 128], AF.Copy)
                      # Wmix groups -> A2 node-major (padded-accumulate trick)
                      for grp in range(3):
                          mlist = [3 * grp, 3 * grp + 1, 3 * grp + 2]
                          for hh in range(2):
                              sW = ndp.tile([96, 512], f32, tag="sW")
                              for ji, m in enumerate(mlist):
                                  b0 = 32 * (m % 3)
                                  pmt = pms.tile([128, 512], f32, tag="pm")
                                  nc.tensor.matmul(
                                      pmt[0:96, :],
                                      sb["WmixP"][b0:b0 + 32, it, m, :],
                                      AT[b0:b0 + 32,
                                         grp, hh * 4:hh * 4 + 4, :]
                                      .rearrange("c w n -> c (w n)"),
                                      start=True, stop=True)
                                  nc.scalar.activation(sW[b0:b0 + 32, :],
                                                       pmt[b0:b0 + 32, :],
                                                       AF.Copy)
                              pmt2 = pms.tile([128, 512], f32, tag="pm")
                              pT = pmt2[:, 0:384]
                              for j4 in range(4):
                                  nc.tensor.transpose(
                                      pT[:, j4 * 96:(j4 + 1) * 96],
                                      sW[:, j4 * 128:(j4 + 1) * 128],
                                      sb["ident"][0:96, 0:96])
                              for j4 in range(4):
                                  w = hh * 4 + j4
                                  nc.scalar.activation(
                                      A2[:, w, grp * 96:(grp + 1) * 96],
                                      pT[:, j4 * 96:(j4 + 1) * 96], AF.Copy)
                      # self-connection
                      if it == 0:
                          scN = sc0n
                      else:
                          scF = nd1.tile([128, WPC, MC], f32, tag="scF")
                          for grp in range(3):
                              mlist = [3 * grp, 3 * grp + 1, 3 * grp + 2]
                              for hh in range(2):
                                  sW = ndp.tile([96, 512], f32, tag="sWs")
                                  for ji, m in enumerate(mlist):
                                      b0 = 32 * (m % 3)
                                      pmt = pms.tile([128, 512], f32, tag="pm")
                                      nc.tensor.matmul(
                                          pmt[0:96, :], WscPb[b0:b0 + 32, m, :],
                                          hTf[b0:b0 + 32,
                                              grp, hh * 4:hh * 4 + 4, :]
                                          .rearrange("c w n -> c (w n)"),
                                          start=True, stop=True)
                                      nc.scalar.activation(sW[b0:b0 + 32, :],
                                                           pmt[b0:b0 + 32, :],
                                                           AF.Copy)
                                  pmt2 = pms.tile([128, 512], f32, tag="pm")
                                  pT = pmt2[:, 0:384]
                                  for j4 in range(4):
                                      nc.tensor.transpose(
                                          pT[:, j4 * 96:(j4 + 1) * 96],
                                          sW[:, j4 * 128:(j4 + 1) * 128],
                                          sb["ident"][0:96, 0:96])
                                  for j4 in range(4):
                                      w = hh * 4 + j4
                                      nc.scalar.activation(
                                          scF[:, w, grp * 96:(grp + 1) * 96],
                                          pT[:, j4 * 96:(j4 + 1) * 96], AF.Copy)
                          scN = scF
                      # F and h update (node-major)
                      F = nd1.tile([128, WPC, C], f32, tag="F")
                      s_ = A2[:, :, 0:C]
                      wz = w123[:, :, it, :]
                      nc.vector.tensor_tensor(F[:], wz[:, :, 64:96], s_, ALU.mult)
                      nc.vector.tensor_tensor(F[:], F[:], wz[:, :, 32:64], ALU.add)
                      nc.vector.tensor_tensor(F[:], F[:], s_, ALU.mult)
                      nc.vector.tensor_tensor(F[:], F[:], wz[:, :, 0:32], ALU.add)
                      nc.vector.tensor_tensor(
                          hN[:].rearrange("p w (m c) -> p w m c", m=NSH),
                          A2[:].rearrange("p w (m c) -> p w m c", m=NSH),
                          F[:].unsqueeze(2).broadcast_to([128, WPC, NSH, C]),
                          ALU.mult)
                      if it == 0:
                          nc.vector.tensor_tensor(hN[:, :, 0:C], hN[:, :, 0:C],
                                                  scN[:], ALU.add)
                      else:
                          nc.vector.tensor_tensor(hN[:], hN[:], scN[:], ALU.add)

                      if KILVL < 3:
                          continue
                      # readouts via stt accumulation
                      scr = ndp.tile([128, 96], f32, tag="scr")
                      base = 1 if it == 0 else 4
                      for w in range(WPC):
                          if it == 0:
                              nc.vector.scalar_tensor_tensor(
                                  scr[:, 0:C], hN[:, w, 0:C], 1.0, sb["wE1bc"][:],
                                  ALU.mult, ALU.mult,
                                  accum_out=vals[:, w, 1].unsqueeze(1))
                          wD = sb["wD1bc"] if it == 0 else sb["wD2bc"]
                          for j in range(3):
                              nc.vector.scalar_tensor_tensor(
                                  scr[:, 0:C], hN[:, w, (1 + j) * C:(2 + j) * C],
                                  1.0, wD[:, j * C:(j + 1) * C], ALU.mult, ALU.mult,
                                  accum_out=vals[:, w, base + 1 + j].unsqueeze(1))

                      if it == 0:
                          # transpose h for iter-1 self-connection
                          for w in range(WPC):
                              pmt = pms.tile([128, 512], f32, tag="pm")
                              pTh = pmt[0:96, 0:384]
                              for grp in range(3):
                                  nc.tensor.transpose(
                                      pTh[:, grp * 128:(grp + 1) * 128],
                                      hN[:, w, grp * 96:(grp + 1) * 96],
                                      sb["ident"][:])
                              for grp in range(3):
                                  nc.scalar.activation(
                                      hTf[:, grp, w, :],
                                      pTh[:, grp * 128:(grp + 1) * 128], AF.Copy)
                          # AllGather scalar channel; agout doubles as gather table
                          agsb = ndp.tile([128, WPC, C], bf16, tag="agsb")
                          nc.scalar.activation(agsb[:], hN[:, :, 0:C], AF.Copy)
                          nc.sync.dma_start(
                              out=agin.ap()[:, 0:C]
                                  .rearrange("(w p) c -> p w c", p=128),
                              in_=agsb[:])
                          if num_devices > 1 and not os.environ.get("KNOAG"):
                              nc.gpsimd.collective_compute(
                                  "AllGather", ALU.bypass,
                                  replica_groups=[list(range(num_devices))],
                                  ins=[agin.ap()], outs=[agout.ap()])
                              gsrc = agout
                          elif num_devices > 1:
                              nc.sync.dma_start(out=agout.ap()[0:NPC, :],
                                                in_=agin.ap())
                              gsrc = agout
                          else:
                              nc.sync.dma_start(out=agout.ap()[0:NPC, :],
                                                in_=agin.ap())
                              gsrc = agout
                          if not os.environ.get("KNOGA"):
                              gather(hs_sb, gsrc, sb["gsnd"], ncol=64)
                      else:
                          # iter-1 energy head: hid = silu(h0 @ Wh); e2 = hid@wE2
                          for w in range(WPC):
                              pmt = pms.tile([128, 512], f32, tag="pm")
                              pTh = pmt[0:32, 0:128]
                              nc.tensor.transpose(pTh[:], hN[:, w, 0:C],
                                                  sb["ident"][:])
                              b0 = 32 * (w % 3)
                              nc.scalar.activation(hT0[b0:b0 + 32, w // 3, :],
                                                   pTh[:], AF.Copy)
                          for w in range(WPC):
                              pmt = pms.tile([128, 512], f32, tag="pm")
                              phid = pmt[0:16, 0:128]
                              b0 = 32 * (w % 3)
                              nc.tensor.matmul(
                                  phid[:], sb["Wh"][b0:b0 + 32, :],
                                  hT0[b0:b0 + 32, w // 3, :],
                                  start=True, stop=True)
                              hidT = ndp.tile([16, 128], f32, tag="hidT")
                              silu(hidT[:], phid[:], ndp)
                              pmt2 = pms.tile([128, 512], f32, tag="pm")
                              pThd = pmt2[:, 0:16]
                              nc.tensor.transpose(pThd[:], hidT[:],
                                                  sb["ident"][0:16, 0:16])
                              hidn = ndp.tile([128, 16], f32, tag="hidn")
                              nc.scalar.activation(hidn[:], pThd[:], AF.Copy)
                              nc.vector.scalar_tensor_tensor(
                                  scr[:, 0:16], hidn[:], 1.0, sb["wE2bc"][:],
                                  ALU.mult, ALU.mult,
                                  accum_out=e2n[:, w].unsqueeze(1))

                  # ---------------- final reduction ----------------
                  if niter >= 1:
                      vc = ndp.tile([128, WPC, 4], f32, tag="vc")
                      nc.vector.tensor_tensor(vc[:, :, 0].unsqueeze(2),
                                              vals[:, :, 0].unsqueeze(2),
                                              vals[:, :, 1].unsqueeze(2), ALU.add)
                      if niter >= 2:
                          nc.vector.tensor_tensor(vc[:, :, 0].unsqueeze(2),
                                                  vc[:, :, 0].unsqueeze(2),
                                                  e2n[:].unsqueeze(2), ALU.add)
                          nc.vector.tensor_tensor(vc[:, :, 1:4], vals[:, :, 2:5],
                                                  vals[:, :, 5:8], ALU.add)
                      else:
                          nc.vector.tensor_copy(vc[:, :, 1:4], vals[:, :, 2:5])
                  else:
                      vc = ndp.tile([128, WPC, 4], f32, tag="vc")
                      nc.vector.tensor_copy(vc[:, :, 0].unsqueeze(2),
                                            vals[:, :, 0].unsqueeze(2))
                      nc.vector.memset(vc[:, :, 1:4], 0.0)
                  for w in range(WPC):
                      nc.vector.scalar_tensor_tensor(
                          vc[:, w, 1:4], sb["posown"][:, w, :],
                          sb["qown"][:, w].unsqueeze(1), vc[:, w, 1:4],
                          ALU.mult, ALU.add)
                  pmt = pms.tile([128, 512], f32, tag="pm")
                  pO = pmt[0:G, 0:4]
                  for w in range(WPC):
                      nc.tensor.matmul(pO[:], sb["goh"][:, w, :], vc[:, w, :],
                                       start=(w == 0), stop=(w == WPC - 1))
                  y_sb = ndp.tile([G, 4], f32, tag="ysb")
                  nc.scalar.activation(y_sb[:], pO[:], AF.Copy)
                  nc.sync.dma_start(out=y_out.ap(), in_=y_sb[:])

    nc.compile()
    return nc


# ----------------------------------------------------------------------------
from concourse.bass_utils import run_bass_kernel_spmd as _run_spmd

_NC_CACHE = {}


def _get_nc():
    if "nc" not in _NC_CACHE:
        _NC_CACHE["nc"] = build_nc(num_devices=NCORES, sim_safe=False,
                                   use_f32r=True)
    return _NC_CACHE["nc"]


def kernel(**inputs):
    np_inputs = {k: np.asarray(v) for k, v in inputs.items()}
    in_maps, _ = host_prep(np_inputs)
    nc = _get_nc()
    res = _run_spmd(nc, in_maps, core_ids=list(range(NCORES)))
    y = sum(np.asarray(res.results[k]["y"], dtype=np.float64)
            for k in range(NCORES))
    return y.astype(np.float32)
